# revision 27
# baseline (speedup 1.0000x reference)
"""Multi-head causal attention block on 8 TRN2 NeuronCores.

Strategy: 8-way tensor parallel over heads (2 heads/core, both batch rows
on every core), all matmuls fp32r (full PE rate needs K=128 and M=128).
Per core:
  phase 1: project x -> kT (both heads stacked on partitions), qT (two
           zero-padded copies so score matmuls can contract K=128 with the
           other head's rows zeroed), v token-major via PE transpose.
  phase 2: causal attention in S^T orientation:
           S^T_h[j,i] = sum_k kT[k,j] * qTz_h[k,i]   (K=128, shared lhsT)
           P^T = exp(S^T) (1/sqrt(dh) folded into wq/bq host-side),
           causal 0/1 mask on diagonal tiles,
           O^T accumulated in PSUM with lhsT = v tile padded to 128 cols
           ([V_h | ones | junk]); row 64 = softmax denominator for free,
           rows 65-127 junk. Normalize O^T rows 0-63 by broadcasted
           1/denominator (PE K=1 broadcast matmul).
  phase 3: AllToAll (8 ranks) turns per-core channel slices into per-core
           token slices; dense2 with full W2, bias b2' = b1v @ W2 + b2
           fused via a K=1 ones matmul.
Output: core c returns tokens [c%4*512:(c%4+1)*512) of batch c//4.
"""

import sys

if "/opt/trn_rl_repo" not in sys.path:
    sys.path.insert(0, "/opt/trn_rl_repo")

import numpy as np

import concourse.bass as bass
import concourse.mybir as mybir
import concourse.tile as tile
from concourse import bacc
from concourse.bass_utils import run_bass_kernel_spmd

F32 = mybir.dt.float32
F32R = mybir.dt.float32r
AF = mybir.ActivationFunctionType

B, T, D = 2, 2048, 1024
NHEADS, DH = 16, 64
NCORE = 8
TT = B * T            # 4096 global token rows
NCHUNK = 8            # 512-token chunks
NTILE = 32            # 128-token tiles


def build_nc():
    nc = bacc.Bacc(
        "TRN2",
        target_bir_lowering=False,
        debug=False,
        enable_asserts=True,
        num_devices=NCORE,
    )
    # ---- DRAM I/O (per core) ----
    xT_d = nc.dram_tensor("xT", [D, TT], F32R, kind="ExternalInput")
    wk_d = nc.dram_tensor("wk", [128, 8, 128], F32R, kind="ExternalInput")
    wq_d = nc.dram_tensor("wq", [128, 8, 128], F32R, kind="ExternalInput")
    wv_d = nc.dram_tensor("wv", [128, 8, 128], F32R, kind="ExternalInput")
    bk_d = nc.dram_tensor("bk", [128, 1], F32, kind="ExternalInput")
    bq_d = nc.dram_tensor("bq", [128, 1], F32, kind="ExternalInput")
    w2_d = nc.dram_tensor("w2", [128, 8, D], F32R, kind="ExternalInput")
    b2p_d = nc.dram_tensor("b2p", [1, D], F32R, kind="ExternalInput")
    masks_d = nc.dram_tensor("masks", [128, 4 * 512], F32R, kind="ExternalInput")
    ones_d = nc.dram_tensor("ones", [1, 128], F32R, kind="ExternalInput")
    ident_d = nc.dram_tensor("ident", [128, 128], F32R, kind="ExternalInput")
    out_d = nc.dram_tensor("out", [512, D], F32, kind="ExternalOutput")

    with tile.TileContext(nc) as tc, nc.allow_low_precision(reason="fp32r pipeline"):
        with (
            tc.tile_pool(name="const", bufs=1) as const,
            tc.tile_pool(name="kq", bufs=1) as kqp,
            tc.tile_pool(name="vp", bufs=1) as vp,
            tc.tile_pool(name="dram", bufs=1, space="DRAM") as dram,
        ):
            # ---- constants (bulky w2/masks DMAs are emitted after phase 1
            # so they don't compete with the xT stream at kernel start) ----
            masks_sb = const.tile([128, 4 * 512], F32R)
            ones_sb = const.tile([1, 128], F32R)
            onesf_sb = const.tile([1, 128], F32)
            ident_sb = const.tile([128, 128], F32R)
            w2_sb = const.tile([128, 8, D], F32R)
            b2p_sb = const.tile([1, D], F32R)
            bk_sb = const.tile([128, 1], F32)
            bq_sb = const.tile([128, 1], F32)
            nc.sync.dma_start(ident_sb[:], ident_d[:])
            nc.sync.dma_start(bk_sb[:], bk_d[:])
            nc.sync.dma_start(bq_sb[:], bq_d[:])

            # ---- persistent activations ----
            kT_sb = kqp.tile([128, TT], F32R)    # rows 0-63 h0, 64-127 h1
            qT_z0 = kqp.tile([128, TT], F32R)    # rows 64-127 zero
            qT_z1 = kqp.tile([128, TT], F32R)    # rows 0-63 zero
            nc.gpsimd.memset(qT_z0[64:128, :].bitcast(F32), 0.0)
            nc.gpsimd.memset(qT_z1[0:64, :].bitcast(F32), 0.0)
            # v token-major, per 128-token tile: cols 0-63 V_h0, 64 ones,
            # 65-128 V_h1, 129 ones, 130-192 junk (PV lhsT is padded to
            # 128 cols: h slice = [65h, 65h+128))
            v_sb = vp.tile([128, NTILE, 193], F32R)
            nc.gpsimd.memset(v_sb[:].bitcast(F32), 1.0)

            # ---- phase 1: projections ----
            with (
                tc.tile_pool(name="wslice", bufs=1) as wsl,
                tc.tile_pool(name="xin", bufs=3) as xin,
                tc.tile_pool(name="vtc", bufs=2) as vtc,
                tc.tile_pool(name="ps1", bufs=2, space="PSUM") as ps1,
                tc.tile_pool(name="pst", bufs=2, space="PSUM") as pst,
            ):
                wk_sb = wsl.tile([128, 8, 128], F32R)
                wq_sb = wsl.tile([128, 8, 128], F32R)
                wv_sb = wsl.tile([128, 8, 128], F32R)
                nc.sync.dma_start(wk_sb[:], wk_d[:])
                nc.sync.dma_start(wq_sb[:], wq_d[:])
                nc.sync.dma_start(wv_sb[:], wv_d[:])

                xT_r = xT_d.ap().rearrange("(a p) t -> p a t", p=128)

                def emit_transposes(vt_c, i8):
                    for t4 in range(4):
                        ps_tr = pst.tile([128, 128], F32R, tag="tp")
                        nc.tensor.transpose(ps_tr[:], vt_c[:, bass.ts(t4, 128)],
                                            ident_sb[:])
                        vi = i8 * 4 + t4
                        nc.vector.tensor_copy(v_sb[:, vi, 0:64], ps_tr[:, 0:64])
                        nc.vector.tensor_copy(v_sb[:, vi, 65:129], ps_tr[:, 64:128])

                pending_vt = None  # transpose chunk i8-1 during chunk i8's MMs
                for i8 in range(NCHUNK):
                    tsl = bass.ts(i8, 512)
                    # two half-chunk tiles so matmuls on the first half can
                    # start while the second half is still streaming in
                    xta = xin.tile([128, 4, 512], F32R, tag="xta")
                    xtb = xin.tile([128, 4, 512], F32R, tag="xtb")
                    nc.sync.dma_start(xta[:], xT_r[:, 0:4, tsl])
                    nc.sync.dma_start(xtb[:], xT_r[:, 4:8, tsl])

                    def xt(a):
                        return xta[:, a, :] if a < 4 else xtb[:, a - 4, :]

                    # kT
                    psk = ps1.tile([128, 512], F32, tag="proj")
                    for a in range(8):
                        nc.tensor.matmul(psk[:], lhsT=wk_sb[:, a, :], rhs=xt(a),
                                         start=(a == 0), stop=(a == 7))
                    nc.scalar.activation(kT_sb[:, tsl], psk[:], AF.Identity,
                                         bias=bk_sb[:], scale=1.0)
                    # qT -> two zero-padded copies (wq/bq pre-scaled on host)
                    psq = ps1.tile([128, 512], F32, tag="proj")
                    for a in range(8):
                        nc.tensor.matmul(psq[:], lhsT=wq_sb[:, a, :], rhs=xt(a),
                                         start=(a == 0), stop=(a == 7))
                    nc.scalar.activation(qT_z0[0:64, tsl], psq[0:64, :], AF.Identity,
                                         bias=bq_sb[0:64, :], scale=1.0)
                    nc.scalar.activation(qT_z1[64:128, tsl], psq[64:128, :],
                                         AF.Identity, bias=bq_sb[64:128, :], scale=1.0)
                    # vT (channel-major) then PE-transpose to token-major
                    psv = ps1.tile([128, 512], F32, tag="proj")
                    for a in range(8):
                        nc.tensor.matmul(psv[:], lhsT=wv_sb[:, a, :], rhs=xt(a),
                                         start=(a == 0), stop=(a == 7))
                    vt_c = vtc.tile([128, 512], F32R, tag="vt")
                    nc.scalar.copy(vt_c[:], psv[:])
                    if pending_vt is not None:
                        emit_transposes(*pending_vt)
                    pending_vt = (vt_c, i8)
                emit_transposes(*pending_vt)

            # bulky constants for later phases — DMA'd while phase 1 computes
            nc.sync.dma_start(masks_sb[:], masks_d[:])
            nc.sync.dma_start(ones_sb[:], ones_d[:])
            nc.sync.dma_start(onesf_sb[:], ones_d[:].bitcast(F32))
            nc.sync.dma_start(w2_sb[:], w2_d[:])
            nc.sync.dma_start(b2p_sb[:], b2p_d[:])

            a2a_send = dram.tile([8, 128, 512], F32R)
            a2a_recv = dram.tile([8, 128, 512], F32R)
            norm_dram = dram.tile([8, 2, 512], F32)  # 1/denominator rows

            # ---- phase 2: attention ----
            qT_z = [qT_z0, qT_z1]
            with (
                tc.tile_pool(name="pp", bufs=6) as pp,
                tc.tile_pool(name="otp", bufs=2) as otp,
                tc.tile_pool(name="bcp", bufs=2) as bcp,
                tc.tile_pool(name="rcp", bufs=2) as rcp,
                tc.tile_pool(name="pss", bufs=4, space="PSUM") as pss,
                tc.tile_pool(name="pso", bufs=4, space="PSUM") as pso,
            ):
                def emit_pv(p_pair, b, kj, po, nkj):
                    for h in range(2):
                        nc.tensor.matmul(
                            po[h][:],
                            lhsT=v_sb[:, b * 16 + kj, 65 * h:65 * h + 128],
                            rhs=p_pair[h][:],
                            start=(kj == 0), stop=(kj == nkj - 1),
                            skip_group_check=True,
                        )

                def emit_norm(po, blk):
                    # normalize O^T rows 0-63 by 1/denominator (row 64):
                    # 1/d = exp(-ln(d)) on ScalarE, then broadcast the row
                    # across 64 partitions with a zero-stride DMA through a
                    # DRAM bounce — the PE is not involved at all, and the
                    # whole chain is deferred into the NEXT block's compute
                    for h in range(2):
                        ld = rcp.tile([1, 512], F32, tag="ld")
                        nc.scalar.activation(ld[:], po[h][64:65, :], AF.Ln)
                        rc = rcp.tile([1, 512], F32, tag="rc")
                        nc.scalar.activation(rc[:], ld[:], AF.Exp, scale=-1.0)
                        # copy O^T out of PSUM right away so the po bank is
                        # released before the DMA broadcast round-trip
                        ou = otp.tile([64, 512], F32, tag="ou")
                        nc.vector.tensor_copy(ou[:], po[h][0:64, :])
                        nc.sync.dma_start(norm_dram[blk, h], rc[:])
                        row = norm_dram[blk, h]
                        bc = bcp.tile([64, 512], F32, tag="bcs")
                        nc.sync.dma_start(
                            bc[:], bass.AP(row.tensor, row.offset, [[0, 64], [1, 512]]))
                        ot = otp.tile([64, 512], F32R, tag="ot")
                        nc.vector.tensor_mul(ot[:], ou[:], bc[:])
                        nc.sync.dma_start(
                            a2a_send[blk, 64 * h:64 * h + 64, :], ot[:])

                pending_norm = None  # previous block's (po, blk)
                for b in range(B):
                    for qi in range(4):
                        qoff = b * T + qi * 512
                        nkj = 4 * qi + 4
                        po0 = pso.tile([128, 512], F32, tag="o")
                        po1 = pso.tile([128, 512], F32, tag="o")
                        po = [po0, po1]
                        pv_queue = []  # PV runs two kj behind S/exp
                        for kj in range(nkj):
                            koff = b * T + kj * 128
                            dp = kj - 4 * qi  # >=0: diagonal tile index
                            p_pair = []
                            for h in range(2):
                                ss = pss.tile([128, 512], F32, tag="s")
                                nc.tensor.matmul(
                                    ss[:],
                                    lhsT=kT_sb[:, koff:koff + 128],
                                    rhs=qT_z[h][:, qoff:qoff + 512],
                                    start=True, stop=True,
                                )
                                p = pp.tile([128, 512], F32R, tag="p")
                                nc.scalar.activation(p[:], ss[:], AF.Exp)
                                if dp >= 0:
                                    nc.vector.tensor_mul(
                                        p[:], p[:], masks_sb[:, bass.ts(dp, 512)])
                                p_pair.append(p)
                            pv_queue.append((p_pair, b, kj, po, nkj))
                            if len(pv_queue) > 2:
                                emit_pv(*pv_queue.pop(0))
                            if kj == 3 and pending_norm is not None:
                                emit_norm(*pending_norm)
                                pending_norm = None
                        for ppv in pv_queue:
                            emit_pv(*ppv)
                        if pending_norm is not None:
                            emit_norm(*pending_norm)
                        pending_norm = (po, b * 4 + qi)
                emit_norm(*pending_norm)

            nc.gpsimd.collective_compute(
                "AllToAll",
                mybir.AluOpType.bypass,
                replica_groups=[list(range(NCORE))],
                ins=[a2a_send.opt()],
                outs=[a2a_recv.opt()],
            )

            # ---- phase 3: dense2 on own 512-token slice ----
            with (
                tc.tile_pool(name="osb", bufs=3) as osbp,
                tc.tile_pool(name="obp", bufs=3) as obp,
                tc.tile_pool(name="psd", bufs=4, space="PSUM") as psd,
            ):
                slabs = []
                for a in range(8):
                    sl = osbp.tile([128, 512], F32R, tag=f"slab{a}")
                    nc.sync.dma_start(sl[:], a2a_recv[a])
                    slabs.append(sl)
                for t4 in range(4):
                    pd0 = psd.tile([128, 512], F32, tag="d")
                    pd1 = psd.tile([128, 512], F32, tag="d")
                    pd = [pd0, pd1]
                    for a in range(8):
                        # both n-halves back-to-back: shared lhsT load
                        for n2 in range(2):
                            nc.tensor.matmul(
                                pd[n2][:],
                                lhsT=slabs[a][:, bass.ts(t4, 128)],
                                rhs=w2_sb[:, a, bass.ts(n2, 512)],
                                start=(a == 0), stop=False,
                                skip_group_check=True,
                            )
                    for n2 in range(2):
                        nsl = bass.ts(n2, 512)
                        nc.tensor.matmul(pd[n2][:], lhsT=ones_sb[:, :],
                                         rhs=b2p_sb[:, nsl],
                                         start=False, stop=True,
                                         skip_group_check=True)
                        ob = obp.tile([128, 512], F32, tag="ob")
                        nc.vector.tensor_copy(ob[:], pd[n2][:])
                        nc.sync.dma_start(out_d[bass.ts(t4, 128), nsl], ob[:])

    nc.compile()
    return nc


_NC_CACHE = {}


def get_nc():
    if "nc" not in _NC_CACHE:
        _NC_CACHE["nc"] = build_nc()
    return _NC_CACHE["nc"]


def make_in_maps(x, W1, b1, W2, b2):
    x = np.asarray(x, dtype=np.float32)
    W1 = np.asarray(W1, dtype=np.float32)
    b1 = np.asarray(b1, dtype=np.float32)
    W2 = np.asarray(W2, dtype=np.float32)
    b2 = np.asarray(b2, dtype=np.float32)

    scale = np.float32(1.0 / np.sqrt(DH))
    xT = np.ascontiguousarray(x.reshape(TT, D).T)  # [D, TT]
    Wk, Wq, Wv = W1[:, :D], W1[:, D:2 * D], W1[:, 2 * D:]
    bk, bq, bv = b1[:D], b1[D:2 * D], b1[2 * D:]
    b2p = (bv @ W2 + b2).reshape(1, D).astype(np.float32)

    # causal masks for the 4 diagonal positions of a [128k x 512q] tile
    j = np.arange(128)[:, None]
    il = np.arange(512)[None, :]
    masks = np.concatenate(
        [(il >= p * 128 + j).astype(np.float32) for p in range(4)], axis=1)

    ones = np.ones((1, 128), np.float32)
    ident = np.eye(128, dtype=np.float32)

    def stack(w):  # [1024, m] -> [128, 8, m] with [p, a, :] = w[a*128+p]
        return np.ascontiguousarray(
            w.reshape(8, 128, -1).transpose(1, 0, 2))

    w2s = stack(W2)
    in_maps = []
    for c in range(NCORE):
        csl = slice(c * 128, (c + 1) * 128)
        in_maps.append({
            "xT": xT,
            "wk": stack(Wk[:, csl]),
            "wq": stack(Wq[:, csl] * scale),
            "wv": stack(Wv[:, csl]),
            "bk": bk[csl].reshape(128, 1).copy(),
            "bq": (bq[csl] * scale).reshape(128, 1).copy(),
            "w2": w2s,
            "b2p": b2p,
            "masks": masks,
            "ones": ones,
            "ident": ident,
        })
    return in_maps


def assemble(results):
    out = np.empty((B, T, D), dtype=np.float32)
    for c in range(NCORE):
        b, t0 = c // 4, (c % 4) * 512
        out[b, t0:t0 + 512, :] = results[c]["out"]
    return out


def kernel(x, W1, b1, W2, b2, _trace=False):
    nc = get_nc()
    in_maps = make_in_maps(x, W1, b1, W2, b2)
    res = run_bass_kernel_spmd(
        nc, in_maps, core_ids=list(range(NCORE)), trace=_trace)
    out = assemble(res.results)
    if _trace:
        return out, res
    return out
